# revision 1
# baseline (speedup 1.0000x reference)
"""AttentionBlock Trainium2 kernel — 8-core SPMD, v6.

vs v5: attention software-pipelined one unit deep: scores+exp of
head-unit u+1 are EMITTED before the consumption tail (denominator,
attn@v, merge) of unit u, so the compile-time engine FIFOs overlap the
slow Act/DVE exp stream of the next unit with the PE tail of the
current one (the scheduler's cost model underestimates Act/DVE 4x and
otherwise serializes units).
"""

import numpy as np
import concourse.bass as bass
import concourse.bacc as bacc
import concourse.mybir as mybir
import concourse.tile as tile
from concourse import bass_utils
from concourse.masks import make_identity

P = 128
N = 2048
D = 512
H2 = 2
DH = 64
DV = 512
E = 2048
QB = 512
NQB = N // QB
NRT = N // P
KC = D // P
EC = E // P
EPS = 1e-5
SCALE = DH ** -0.5

f32 = mybir.dt.float32
f32r = mybir.dt.float32r
bf16 = mybir.dt.bfloat16
i16 = mybir.dt.int16
A16 = 184.6650085170266   # 2^7/ln2
B16 = 16249.0             # 127*2^7 + schraudolph offset (tuned)

AF = mybir.ActivationFunctionType
ALU = mybir.AluOpType


def build_body(tc, ins, outs, fake_rs=False, phases="ABC", flags=()):
    nc = tc.nc
    x, xrb, wqkv, bq_pt, wm, w1, b1_pt, w2, b2_d = ins
    out = outs["out"]
    halve_scores = "scores8" in flags
    halve_av = "av8" in flags
    no_dn = "nodn" in flags
    no_av = "noav" in flags
    no_exp = "noexp" in flags
    no_merge = "nomerge" in flags
    act_silu = "exp_silu" not in flags
    act_sqrt2 = "newton_ln2" not in flags

    import contextlib
    est = contextlib.ExitStack()
    with est:
        const = est.enter_context(tc.tile_pool(name="const", bufs=1))
        dram = est.enter_context(tc.tile_pool(name="dram", bufs=1, space="DRAM"))
        main = est.enter_context(tc.tile_pool(name="main", bufs=1))
        stream = est.enter_context(tc.tile_pool(name="stream", bufs=3))
        psum = est.enter_context(tc.tile_pool(name="psum", bufs=1, space="PSUM"))

        # ---- constants ----
        ident_f = const.tile([P, P], f32)
        make_identity(nc, ident_f)
        ident = const.tile([P, P], bf16)
        nc.vector.tensor_copy(ident, ident_f)
        ones_f = const.tile([P, 1], f32)
        nc.vector.memset(ones_f, 1.0)
        ones_col = const.tile([P, 1], bf16)
        nc.vector.tensor_copy(ones_col, ones_f)
        ones_rf = const.tile([1, P], f32)
        nc.vector.memset(ones_rf, 1.0)
        ones_row = const.tile([1, P], f32r)
        nc.vector.tensor_copy(ones_row, ones_rf)
        eps_t = const.tile([P, 1], f32)
        nc.vector.memset(eps_t, EPS)

        bq_sb = const.tile([P, 1], f32)
        nc.sync.dma_start(out=bq_sb, in_=bq_pt[:, :])
        b1_sb = const.tile([P, EC], f32)
        nc.sync.dma_start(out=b1_sb, in_=b1_pt[:, :])
        nb1_sb = const.tile([P, EC], f32)
        nc.vector.tensor_scalar(out=nb1_sb, in0=b1_sb, scalar1=-1.0, scalar2=None,
                                op0=ALU.mult)
        b2_b = const.tile([P, D], f32)
        nc.gpsimd.dma_start(
            out=b2_b,
            in_=bass.AP(tensor=b2_d.tensor, offset=b2_d.offset, ap=[[0, P], [1, D]]))

        dummy_d = dram.tile([P, QB], bf16, name="dummy_d", tag="dummy_d")
        rs_in = [dram.tile([QB, D], bf16, name=f"rs_in{j}", tag=f"rs_in{j}")
                 for j in range(NQB)]
        rs_out = [dram.tile([P, D], bf16, name=f"rs_out{j}", tag=f"rs_out{j}")
                  for j in range(NQB)]

        # ---- persistent SBUF tiles (weights on the gpsimd SW queue) ----
        wqkv_sb = main.tile([P, KC, 2 * H2 * DH + H2 * DV], bf16)
        nc.gpsimd.dma_start(out=wqkv_sb, in_=wqkv.rearrange("(c p) n -> p c n", p=P))
        wm_sb = main.tile([P, H2 * DV // P, D], bf16)
        nc.gpsimd.dma_start(out=wm_sb, in_=wm.rearrange("(c p) n -> p c n", p=P))
        w1_sb = main.tile([P, KC, E], bf16)
        w1r = w1.rearrange("(c p) n -> p c n", p=P)
        for kc in range(KC):
            nc.gpsimd.dma_start(out=w1_sb[:, kc, :], in_=w1r[:, kc, :])
        w2_sb = main.tile([P, EC, D], bf16)
        w2r = w2.rearrange("(c p) n -> p c n", p=P)
        for j in range(4):
            nc.gpsimd.dma_start(out=w2_sb[:, 4 * j:4 * (j + 1), :],
                                in_=w2r[:, 4 * j:4 * (j + 1), :])

        qkT = main.tile([P, 2, N], bf16)
        v_sb = main.tile([P, NRT, H2 * DV], bf16)
        x2_sb = main.tile([P, NQB, D], f32)

        # ---------------- Phase A: LN1 + xbar transposes + qkv ----------------
        with tc.tile_pool(name="poolA", bufs=1) as poolA:
            xnT = poolA.tile([P, KC, N], bf16)

            for rt in range(NRT):
                x_t = stream.tile([P, D], bf16, tag="x_t", bufs=3)
                nc.sync.dma_start(out=x_t, in_=x[rt * P:(rt + 1) * P, :])
                st6 = stream.tile([P, 6], f32, tag="st6")
                nc.vector.bn_stats(out=st6, in_=x_t)
                mv = stream.tile([P, 2], f32, tag="mv")
                nc.vector.bn_aggr(out=mv, in_=st6)
                sd = stream.tile([P, 1], f32, tag="sd")
                nc.scalar.activation(out=sd, in_=mv[:, 1:2], func=AF.Sqrt,
                                     bias=eps_t, scale=1.0)
                rstd = stream.tile([P, 1], f32, tag="rstd")
                nc.vector.reciprocal(out=rstd, in_=sd)
                xn_t = stream.tile([P, D], bf16, tag="xn_t", bufs=3)
                nc.vector.tensor_scalar(out=xn_t, in0=x_t,
                                        scalar1=mv[:, 0:1], scalar2=rstd,
                                        op0=ALU.subtract, op1=ALU.mult)
                for kc in range(KC):
                    psT = psum.tile([P, P], bf16, tag="t", bufs=1)
                    nc.tensor.transpose(psT, xn_t[:, kc * P:(kc + 1) * P], ident)
                    nc.vector.tensor_copy(out=xnT[:, kc, rt * P:(rt + 1) * P],
                                          in_=psT)

            for ct in range(2):
                for rr in range(4):
                    ps_qk = psum.tile([P, QB], f32, tag="s", bufs=2)
                    for kc in range(KC):
                        nc.tensor.matmul(
                            ps_qk, wqkv_sb[:, kc, ct * P:(ct + 1) * P],
                            xnT[:, kc, rr * QB:(rr + 1) * QB],
                            start=(kc == 0), stop=(kc == KC - 1))
                    nc.scalar.activation(
                        out=qkT[:, ct, rr * QB:(rr + 1) * QB], in_=ps_qk,
                        func=AF.Identity,
                        bias=(bq_sb if ct == 0 else 0.0), scale=1.0)

            for mt in range(NRT):
                for cr in range(2):
                    ps_v = psum.tile([P, DV], f32, tag="av", bufs=2)
                    for kc in range(KC):
                        nc.tensor.matmul(
                            ps_v, xnT[:, kc, mt * P:(mt + 1) * P],
                            wqkv_sb[:, kc, 2 * H2 * DH + cr * DV:
                                    2 * H2 * DH + (cr + 1) * DV],
                            start=(kc == 0), stop=(kc == KC - 1))
                    nc.vector.tensor_copy(
                        out=v_sb[:, mt, cr * DV:(cr + 1) * DV], in_=ps_v)

        # ---------------- Phase B/C interleaved per q-block ----------------
        # software pipeline state: per-unit (qb, hh) eT tiles
        unit_state = {}

        def scores_exp(qb, hh):
            """Emit scores + exp for unit (qb, hh); eT kept in unit_state."""
            hp = slice(DH * hh, DH * (hh + 1))
            eT = stream.tile([P, NRT, QB], bf16, tag="eT", bufs=2,
                             name=f"eT_{qb}_{hh}")
            for kt in range(NRT):
                ps_s = psum.tile([P, QB], f32, tag="s", bufs=2)
                nc.tensor.matmul(
                    ps_s, qkT[hp, 1, kt * P:(kt + 1) * P],
                    qkT[hp, 0, qb * QB:(qb + 1) * QB],
                    start=True, stop=True)
                if kt % 2 == 0:
                    # scale pre-folded into q on the host
                    nc.scalar.activation(out=eT[:, kt, :], in_=ps_s,
                                         func=AF.Exp, scale=1.0)
                else:
                    # Schraudolph exp: bf16 bits = s*2^7/ln2 + B16
                    with nc.allow_low_precision(reason="schraudolph exp"):
                        nc.vector.tensor_scalar(
                            out=eT[:, kt, :].bitcast(i16), in0=ps_s,
                            scalar1=A16, scalar2=B16,
                            op0=ALU.mult, op1=ALU.add)
            unit_state[(qb, hh)] = eT

        def attn_tail(qb, hh, oT):
            """Denominator, attn@v, oT normalize for unit (qb, hh)."""
            eT = unit_state.pop((qb, hh))
            ps_d = psum.tile([1, QB], f32, tag="dn", bufs=1)
            for kt in range(NRT):
                nc.tensor.matmul(ps_d, ones_col, eT[:, kt, :],
                                 start=(kt == 0), stop=(kt == NRT - 1))
            rd = stream.tile([1, QB], f32r, tag="rd", bufs=2)
            with nc.allow_low_precision(reason="f32r for PE bcast"):
                nc.vector.reciprocal(out=rd, in_=ps_d)
            ps_b = psum.tile([P, QB], f32, tag="m", bufs=2)
            nc.tensor.matmul(ps_b, ones_row, rd, start=True, stop=True)
            bc = stream.tile([P, QB], f32, tag="bc", bufs=2)
            nc.vector.tensor_copy(out=bc, in_=ps_b)
            for ct in range(DV // P):
                ps_av = psum.tile([P, QB], f32, tag="av", bufs=2)
                for mc in range(NRT):
                    nc.tensor.matmul(
                        ps_av,
                        v_sb[:, mc, hh * DV + ct * P:hh * DV + (ct + 1) * P],
                        eT[:, mc, :],
                        start=(mc == 0), stop=(mc == NRT - 1))
                nc.vector.tensor_tensor(
                    out=oT[:, hh * (DV // P) + ct, :], in0=ps_av,
                    in1=bc, op=ALU.mult)

        def merge_rs(qb, oT):
            for qt in range(QB // P):
                ps_m = psum.tile([P, D], f32, tag="m", bufs=2)
                for ch in range(H2 * DV // P):
                    nc.tensor.matmul(
                        ps_m, oT[:, ch, qt * P:(qt + 1) * P], wm_sb[:, ch, :],
                        start=(ch == 0), stop=(ch == H2 * DV // P - 1))
                pt_sb = stream.tile([P, D], bf16, tag="pt_sb", bufs=2)
                nc.vector.tensor_copy(out=pt_sb, in_=ps_m)
                nc.sync.dma_start(out=rs_in[qb][qt * P:(qt + 1) * P, :],
                                  in_=pt_sb)
            if fake_rs:
                nc.sync.dma_start(out=rs_out[qb][:, :], in_=rs_in[qb][0:P, :])
            else:
                nc.gpsimd.collective_compute(
                    "ReduceScatter", ALU.add,
                    replica_groups=[[0, 1, 2, 3], [4, 5, 6, 7]],
                    ins=[rs_in[qb].opt()], outs=[rs_out[qb].opt()])

        def ffn_qb(qb):
            rs_t = stream.tile([P, D], bf16, tag="rs_t", bufs=2)
            nc.sync.dma_start(out=rs_t, in_=rs_out[qb][:, :])
            xr_t = stream.tile([P, D], f32, tag="xr_t", bufs=2)
            nc.sync.dma_start(out=xr_t, in_=xrb[qb, :, :])
            nc.vector.tensor_tensor(out=x2_sb[:, qb, :], in0=rs_t, in1=xr_t,
                                    op=ALU.add)
            st6 = stream.tile([P, 6], f32, tag="st6c")
            nc.vector.bn_stats(out=st6, in_=x2_sb[:, qb, :])
            mv = stream.tile([P, 2], f32, tag="mvc")
            nc.vector.bn_aggr(out=mv, in_=st6)
            rstd = stream.tile([P, 1], f32, tag="rstdc")
            if act_sqrt2:
                sd = stream.tile([P, 1], f32, tag="sdc")
                nc.scalar.activation(out=sd, in_=mv[:, 1:2], func=AF.Sqrt,
                                     bias=eps_t, scale=1.0)
                nc.vector.reciprocal(out=rstd, in_=sd)
            else:
                # Newton rsqrt on DVE: y0 linear seed, 4 rounds of
                # y <- y*(1.5 - 0.5*v*y^2); v = var+eps in ~[0.6, 2.5]
                v_t = stream.tile([P, 1], f32, tag="v_t")
                nc.vector.tensor_scalar(out=v_t, in0=mv[:, 1:2],
                                        scalar1=EPS, scalar2=None, op0=ALU.add)
                y = stream.tile([P, 1], f32, tag="y_t")
                nc.vector.tensor_scalar(out=y, in0=v_t, scalar1=-0.375,
                                        scalar2=1.458, op0=ALU.mult, op1=ALU.add)
                y2 = stream.tile([P, 1], f32, tag="y2_t")
                vy2 = stream.tile([P, 1], f32, tag="vy2_t")
                for _ in range(4):
                    nc.vector.tensor_tensor(out=y2, in0=y, in1=y, op=ALU.mult)
                    nc.vector.tensor_tensor(out=vy2, in0=v_t, in1=y2, op=ALU.mult)
                    nc.vector.tensor_scalar(out=vy2, in0=vy2, scalar1=-0.5,
                                            scalar2=1.5, op0=ALU.mult, op1=ALU.add)
                    nc.vector.tensor_tensor(out=y, in0=y, in1=vy2, op=ALU.mult)
                nc.vector.tensor_copy(out=rstd, in_=y)
            xn2_t = stream.tile([P, D], bf16, tag="xn2_t", bufs=2)
            nc.vector.tensor_scalar(out=xn2_t, in0=x2_sb[:, qb, :],
                                    scalar1=mv[:, 0:1], scalar2=rstd,
                                    op0=ALU.subtract, op1=ALU.mult)
            xn2T = stream.tile([P, KC, P], bf16, tag="xn2T", bufs=2)
            for kc in range(KC):
                psT = psum.tile([P, P], bf16, tag="t", bufs=1)
                nc.tensor.transpose(psT, xn2_t[:, kc * P:(kc + 1) * P], ident)
                nc.vector.tensor_copy(out=xn2T[:, kc, :], in_=psT)

            hT = stream.tile([P, EC, P], bf16, tag="hT", bufs=1)
            for et in range(EC):
                ps_h = psum.tile([P, P], f32, tag="av", bufs=2)
                for kc in range(KC):
                    nc.tensor.matmul(ps_h, w1_sb[:, kc, et * P:(et + 1) * P],
                                     xn2T[:, kc, :],
                                     start=(kc == 0), stop=(kc == KC - 1))
                if act_silu:
                    nc.scalar.activation(out=hT[:, et, :], in_=ps_h, func=AF.Silu,
                                         bias=b1_sb[:, et:et + 1], scale=1.0)
                else:
                    # swish via exp table: t=exp(-(h)); hT = h/(1+t)
                    # t = exp(-(h+b1)) = exp(-h - b1)
                    tneg = stream.tile([P, P], bf16, tag="tneg", bufs=2)
                    nc.scalar.activation(out=tneg, in_=ps_h, func=AF.Exp,
                                         bias=nb1_sb[:, et:et + 1], scale=-1.0)
                    den = stream.tile([P, P], f32, tag="den", bufs=2)
                    nc.vector.tensor_scalar(out=den, in0=tneg, scalar1=1.0,
                                            scalar2=None, op0=ALU.add)
                    rden = stream.tile([P, P], f32, tag="rden", bufs=2)
                    nc.vector.reciprocal(out=rden, in_=den)
                    hb = stream.tile([P, P], f32, tag="hb", bufs=2)
                    nc.vector.tensor_scalar(out=hb, in0=ps_h,
                                            scalar1=b1_sb[:, et:et + 1],
                                            scalar2=None, op0=ALU.add)
                    nc.vector.tensor_tensor(out=hT[:, et, :], in0=hb,
                                            in1=rden, op=ALU.mult)

            ps_o = psum.tile([P, D], f32, tag="s", bufs=2)
            for ec in range(EC):
                nc.tensor.matmul(ps_o, hT[:, ec, :], w2_sb[:, ec, :],
                                 start=(ec == 0), stop=(ec == EC - 1))
            o_t = stream.tile([P, D], f32, tag="o_t", bufs=2)
            nc.vector.tensor_tensor(out=o_t, in0=ps_o, in1=x2_sb[:, qb, :],
                                    op=ALU.add)
            nc.vector.tensor_tensor(out=o_t, in0=o_t, in1=b2_b, op=ALU.add)
            nc.sync.dma_start(out=out[qb * P:(qb + 1) * P, :], in_=o_t)

        # pipelined emission: scores/exp of unit u+1 ahead of tail of u;
        # FF of qb interleaves once its RS has been issued two q-blocks back
        units = [(qb, hh) for qb in range(NQB) for hh in range(H2)]
        oTs = {}
        if phases == "A":
            units = []
        scheduled_ffn = {"AB": [], "ABC": [None, None, 0, None, 1, None, 2, 3],
                         "A": []}[phases if phases in ("A", "AB") else "ABC"]
        if phases == "A":
            pass
        else:
            scores_exp(*units[0])
            for i, (qb, hh) in enumerate(units):
                if hh == 0:
                    oTs[qb] = stream.tile([P, H2 * DV // P, QB], bf16,
                                          tag="oT", bufs=2, name=f"oT{qb}")
                if i + 1 < len(units):
                    scores_exp(*units[i + 1])
                attn_tail(qb, hh, oTs[qb])
                if hh == 1:
                    merge_rs(qb, oTs.pop(qb))
                if phases == "ABC" and scheduled_ffn[i] is not None:
                    ffn_qb(scheduled_ffn[i])


def build_nc(repeat=None, fake_rs=False, phases="ABC", flags=()):
    nc = bacc.Bacc("TRN2", target_bir_lowering=False, debug=False, num_devices=8)
    x = nc.dram_tensor("x", [N, D], bf16, kind="ExternalInput")
    xrb = nc.dram_tensor("xrb", [NQB, P, D], f32, kind="ExternalInput")
    wqkv = nc.dram_tensor("wqkv", [D, 2 * H2 * DH + H2 * DV], bf16,
                          kind="ExternalInput")
    bq_pt = nc.dram_tensor("bq_pt", [P, 1], f32, kind="ExternalInput")
    wm = nc.dram_tensor("wm", [H2 * DV, D], bf16, kind="ExternalInput")
    w1 = nc.dram_tensor("w1", [D, E], bf16, kind="ExternalInput")
    b1_pt = nc.dram_tensor("b1_pt", [P, EC], f32, kind="ExternalInput")
    w2 = nc.dram_tensor("w2", [E, D], bf16, kind="ExternalInput")
    b2 = nc.dram_tensor("b2", [1, D], f32, kind="ExternalInput")

    outs = {"out": nc.dram_tensor("out", [NQB * P, D], f32,
                                  kind="ExternalOutput").ap()}
    ins = (x.ap(), xrb.ap(), wqkv.ap(), bq_pt.ap(), wm.ap(),
           w1.ap(), b1_pt.ap(), w2.ap(), b2.ap())
    with tile.TileContext(nc) as tc:
        if repeat:
            with tc.For_i(0, repeat, 1):
                build_body(tc, ins, outs, fake_rs=fake_rs, phases=phases,
                           flags=flags)
        else:
            build_body(tc, ins, outs, fake_rs=fake_rs, phases=phases,
                       flags=flags)
    nc.compile()
    return nc


def make_in_maps(inputs):
    """Host-side prep: fold LN gains into weights, drop the k-bias (cancels
    in softmax), fold bv@Wm + bm into the residual rows, fold the softmax
    scale into the q weights, convert matmul operands to bf16."""
    from ml_dtypes import bfloat16 as np_bf16

    x = np.asarray(inputs["x"], np.float32)
    ln1_g = np.asarray(inputs["ln1_g"], np.float32)
    ln1_b = np.asarray(inputs["ln1_b"], np.float32)
    Wqkv = np.asarray(inputs["Wqkv"], np.float32)
    bqkv = np.asarray(inputs["bqkv"], np.float32)
    Wm = np.asarray(inputs["Wm"], np.float32)
    bm = np.asarray(inputs["bm"], np.float32)
    ln2_g = np.asarray(inputs["ln2_g"], np.float32)
    ln2_b = np.asarray(inputs["ln2_b"], np.float32)
    W1 = np.asarray(inputs["W1"], np.float32)
    b1 = np.asarray(inputs["b1"], np.float32)
    W2 = np.asarray(inputs["W2"], np.float32)
    b2 = np.asarray(inputs["b2"], np.float32)

    Wqkv_eff = ln1_g[:, None] * Wqkv
    bqkv_eff = ln1_b @ Wqkv + bqkv
    W1_eff = ln2_g[:, None] * W1
    b1_eff = ln2_b @ W1 + b1

    DQ = 512
    bv_full = bqkv_eff[2 * DQ:]
    bvwm = bv_full @ Wm

    in_maps = []
    for c in range(8):
        b = c // 4
        g = c % 4
        qcols = slice(DH * 2 * g, DH * 2 * g + 2 * DH)
        kcols = slice(DQ + DH * 2 * g, DQ + DH * 2 * g + 2 * DH)
        vcols = slice(2 * DQ + H2 * DV * g, 2 * DQ + H2 * DV * (g + 1))
        wqkv_c = np.concatenate(
            [Wqkv_eff[:, qcols] * SCALE, Wqkv_eff[:, kcols],
             Wqkv_eff[:, vcols]], axis=1)
        bq = bqkv_eff[qcols] * SCALE
        wm_c = Wm[H2 * DV * g:H2 * DV * (g + 1), :]
        xrb = np.stack([x[b, QB * j + P * g:QB * j + P * (g + 1), :]
                        for j in range(NQB)]) \
            + bm[None, None, :] + bvwm[None, None, :]
        in_maps.append({
            "x": np.ascontiguousarray(x[b].astype(np_bf16)),
            "xrb": np.ascontiguousarray(xrb.astype(np.float32)),
            "wqkv": np.ascontiguousarray(wqkv_c.astype(np_bf16)),
            "bq_pt": np.ascontiguousarray(bq[:, None].astype(np.float32)),
            "wm": np.ascontiguousarray(wm_c.astype(np_bf16)),
            "w1": np.ascontiguousarray(W1_eff.astype(np_bf16)),
            "b1_pt": np.ascontiguousarray(
                b1_eff.reshape(EC, P).T.astype(np.float32)),
            "w2": np.ascontiguousarray(W2.astype(np_bf16)),
            "b2": np.ascontiguousarray(b2[None, :].astype(np.float32)),
        })
    return in_maps


def assemble_output(results):
    full = np.empty((2, N, D), np.float32)
    for c in range(8):
        b, rank = c // 4, c % 4
        o = results[c]["out"]
        for j in range(NQB):
            full[b, QB * j + P * rank:QB * j + P * (rank + 1), :] = \
                o[P * j:P * (j + 1), :]
    return full

_NC_CACHE = {}


def kernel(**inputs) -> np.ndarray:
    key = "nc8"
    if key not in _NC_CACHE:
        _NC_CACHE[key] = build_nc()
    nc = _NC_CACHE[key]
    in_maps = make_in_maps(inputs)
    res = bass_utils.run_bass_kernel_spmd(nc, in_maps, core_ids=list(range(8)))
    return assemble_output(res.results)



# revision 2
# speedup vs baseline: 1.2256x; 1.2256x over previous
"""AttentionBlock Trainium2 kernel — 8-core SPMD, v7.

vs v6:
- Wv@Wm folded on host per head: v' = xn @ (Wv_h Wm_h) so the merge
  matmul disappears; head outputs are summed after per-head softmax
  normalization (rows of A sum to 1, so bv@Wm still folds into the
  residual).
- attn@v uses eT chunks as the stationary operand, producing [q, dv]
  directly (the layout the ReduceScatter wants) — no output transpose.
- The softmax denominator rides along as a leading ones-column in the
  v' moving operand (PSUM col 0 accumulates sum_k exp), killing the
  separate ones-matmul denominator chain and the bc broadcast.
- exp split 3 ways across Act (table exp) / DVE / Pool (Schraudolph)
  so the PE never waits on PSUM drain during scores.
- LN2 rstd via 2-round Newton on DVE (var+eps in [0.8, 1.21]) so the
  Act engine table never leaves exp<->silu; fewer 1.3us table reloads.
- Phase A: v' projection interleaved per row-tile right after its
  transposes; PSUM->SBUF copies spread across Act/DVE/Pool.
"""

import numpy as np
import concourse.bass as bass
import concourse.bacc as bacc
import concourse.mybir as mybir
import concourse.tile as tile
from concourse import bass_utils
from concourse.masks import make_identity

P = 128
N = 2048
D = 512
H2 = 2
DH = 64
DV = 512           # folded per-head value dim == D
VW = DV + 1        # [ones | v'] per head
E = 2048
QB = 512
NQB = N // QB      # 4
NRT = N // P       # 16
KC = D // P        # 4
EC = E // P        # 16
EPS = 1e-5
SCALE = DH ** -0.5

f32 = mybir.dt.float32
bf16 = mybir.dt.bfloat16
i16 = mybir.dt.int16
A16 = 184.6650085170266   # 2^7/ln2
B16 = 16249.0             # 127*2^7 + schraudolph offset (tuned)

AF = mybir.ActivationFunctionType
ALU = mybir.AluOpType


def build_body(tc, ins, outs, fake_rs=False, phases="ABC", flags=()):
    nc = tc.nc
    x, xrb, wqkv, bq_pt, w1, b1_pt, w2, b2_d = ins
    out = outs["out"]
    exp2 = "exp2" in flags            # v6-style 2-way exp split
    ffn_late = "ffnlate" in flags     # all FFN at the end

    import contextlib
    est = contextlib.ExitStack()
    with est:
        const = est.enter_context(tc.tile_pool(name="const", bufs=1))
        dram = est.enter_context(tc.tile_pool(name="dram", bufs=1, space="DRAM"))
        main = est.enter_context(tc.tile_pool(name="main", bufs=1))
        stream = est.enter_context(tc.tile_pool(name="stream", bufs=3))
        psum = est.enter_context(tc.tile_pool(name="psum", bufs=1, space="PSUM"))

        # ---- constants ----
        ident_f = const.tile([P, P], f32)
        make_identity(nc, ident_f)
        ident = const.tile([P, P], bf16)
        nc.vector.tensor_copy(ident, ident_f)
        eps_t = const.tile([P, 1], f32)
        nc.vector.memset(eps_t, EPS)

        bq_sb = const.tile([P, 1], f32)
        nc.sync.dma_start(out=bq_sb, in_=bq_pt[:, :])
        b1_sb = const.tile([P, EC], f32)
        nc.sync.dma_start(out=b1_sb, in_=b1_pt[:, :])
        b2_b = const.tile([P, D], f32)
        nc.gpsimd.dma_start(
            out=b2_b,
            in_=bass.AP(tensor=b2_d.tensor, offset=b2_d.offset, ap=[[0, P], [1, D]]))

        rs_in = [dram.tile([QB, D], bf16, name=f"rs_in{j}", tag=f"rs_in{j}")
                 for j in range(NQB)]
        rs_out = [dram.tile([P, D], bf16, name=f"rs_out{j}", tag=f"rs_out{j}")
                  for j in range(NQB)]

        # ---- persistent SBUF tiles (weights on the gpsimd SW queue) ----
        wqkv_sb = main.tile([P, KC, 2 * H2 * DH + H2 * DV], bf16)
        nc.gpsimd.dma_start(out=wqkv_sb, in_=wqkv.rearrange("(c p) n -> p c n", p=P))
        w1_sb = main.tile([P, KC, E], bf16)
        w1r = w1.rearrange("(c p) n -> p c n", p=P)
        for kc in range(KC):
            nc.gpsimd.dma_start(out=w1_sb[:, kc, :], in_=w1r[:, kc, :])
        w2_sb = main.tile([P, EC, D], bf16)
        w2r = w2.rearrange("(c p) n -> p c n", p=P)
        for j in range(4):
            nc.gpsimd.dma_start(out=w2_sb[:, 4 * j:4 * (j + 1), :],
                                in_=w2r[:, 4 * j:4 * (j + 1), :])

        qkT = main.tile([P, 2, N], bf16)
        v_sb = main.tile([P, NRT, H2 * VW], bf16)
        x2_sb = main.tile([P, NQB, D], f32)

        # ones columns of v' (denominator rider)
        for hh in range(H2):
            nc.vector.memset(v_sb[:, :, hh * VW:hh * VW + 1], 1.0)

        # ---------------- Phase A: LN1 + xbar transposes + qkv ----------------
        a_inline = "a_inline" in flags    # v' interleaved per row-tile
        a_actcp = "a_actcp" in flags      # split PSUM copies DVE/Act
        a_poolxn = "a_poolxn" in flags    # xn normalize on Pool

        def ln_rt(rt, xnT):
            x_t = stream.tile([P, D], bf16, tag="x_t", bufs=3)
            nc.sync.dma_start(out=x_t, in_=x[rt * P:(rt + 1) * P, :])
            st6 = stream.tile([P, 6], f32, tag="st6")
            nc.vector.bn_stats(out=st6, in_=x_t)
            mv = stream.tile([P, 2], f32, tag="mv")
            nc.vector.bn_aggr(out=mv, in_=st6)
            sd = stream.tile([P, 1], f32, tag="sd")
            nc.scalar.activation(out=sd, in_=mv[:, 1:2], func=AF.Sqrt,
                                 bias=eps_t, scale=1.0)
            rstd = stream.tile([P, 1], f32, tag="rstd")
            nc.vector.reciprocal(out=rstd, in_=sd)
            xn_t = stream.tile([P, D], bf16, tag="xn_t", bufs=3)
            xn_eng = nc.gpsimd if a_poolxn else nc.vector
            xn_eng.tensor_scalar(out=xn_t, in0=x_t,
                                 scalar1=mv[:, 0:1], scalar2=rstd,
                                 op0=ALU.subtract, op1=ALU.mult)
            for kc in range(KC):
                psT = psum.tile([P, P], bf16, tag="t", bufs=1)
                nc.tensor.transpose(psT, xn_t[:, kc * P:(kc + 1) * P], ident)
                dst = xnT[:, kc, rt * P:(rt + 1) * P]
                if a_actcp and rt % 2 == 1:
                    nc.scalar.activation(out=dst, in_=psT, func=AF.Identity,
                                         scale=1.0)
                else:
                    nc.vector.tensor_copy(out=dst, in_=psT)

        def vproj_rt(rt, xnT):
            for cr in range(H2):
                ps_v = psum.tile([P, DV], f32, tag="avA", bufs=2)
                for kc in range(KC):
                    nc.tensor.matmul(
                        ps_v, xnT[:, kc, rt * P:(rt + 1) * P],
                        wqkv_sb[:, kc, 2 * H2 * DH + cr * DV:
                                2 * H2 * DH + (cr + 1) * DV],
                        start=(kc == 0), stop=(kc == KC - 1))
                m = (rt * H2 + cr) % 8
                dst = v_sb[:, rt, cr * VW + 1:(cr + 1) * VW]
                if a_actcp and m >= 3:
                    nc.scalar.activation(out=dst, in_=ps_v, func=AF.Identity,
                                         scale=1.0)
                else:
                    nc.vector.tensor_copy(out=dst, in_=ps_v)

        def qk_rr(rr, xnT):
            for ct in range(2):
                ps_qk = psum.tile([P, QB], f32, tag="s", bufs=3)
                for kc in range(KC):
                    nc.tensor.matmul(
                        ps_qk, wqkv_sb[:, kc, ct * P:(ct + 1) * P],
                        xnT[:, kc, rr * QB:(rr + 1) * QB],
                        start=(kc == 0), stop=(kc == KC - 1))
                nc.scalar.activation(
                    out=qkT[:, ct, rr * QB:(rr + 1) * QB], in_=ps_qk,
                    func=AF.Identity,
                    bias=(bq_sb if ct == 0 else 0.0), scale=1.0)

        with tc.tile_pool(name="poolA", bufs=1) as poolA:
            xnT = poolA.tile([P, KC, N], bf16)
            if a_inline:
                for rt in range(NRT):
                    ln_rt(rt, xnT)
                    vproj_rt(rt, xnT)
                    if rt % 4 == 3:
                        qk_rr(rt // 4, xnT)
            else:
                for rt in range(NRT):
                    ln_rt(rt, xnT)
                for rr in range(NQB):
                    qk_rr(rr, xnT)
                for rt in range(NRT):
                    vproj_rt(rt, xnT)

        # ---------------- Phase B/C interleaved ----------------
        unit_state = {}

        def scores_exp(qb, hh):
            hp = slice(DH * hh, DH * (hh + 1))
            eT = stream.tile([P, NRT, QB], bf16, tag="eT", bufs=2,
                             name=f"eT_{qb}_{hh}")
            for kt in range(NRT):
                ps_s = psum.tile([P, QB], f32, tag="s", bufs=3)
                nc.tensor.matmul(
                    ps_s, qkT[hp, 1, kt * P:(kt + 1) * P],
                    qkT[hp, 0, qb * QB:(qb + 1) * QB],
                    start=True, stop=True)
                if kt % 2 == 0:
                    # scale pre-folded into q on the host
                    nc.scalar.activation(out=eT[:, kt, :], in_=ps_s,
                                         func=AF.Exp, scale=1.0)
                else:
                    # Schraudolph exp: bf16 bits = s*2^7/ln2 + B16
                    with nc.allow_low_precision(reason="schraudolph exp"):
                        nc.vector.tensor_scalar(
                            out=eT[:, kt, :].bitcast(i16), in0=ps_s,
                            scalar1=A16, scalar2=B16,
                            op0=ALU.mult, op1=ALU.add)
            unit_state[(qb, hh)] = eT

        def attn_tail(qb, hh, oacc_qb):
            """A@v' in [q, dv] layout with denominator in PSUM col 0;
            per-head normalize via per-partition scalar; heads summed."""
            eT = unit_state.pop((qb, hh))
            base = hh * VW
            for qc in range(4):
                psA = psum.tile([P, 257], f32, tag="avA", bufs=2)
                psB = psum.tile([P, 256], f32, tag="avB", bufs=2)
                for mc in range(NRT):
                    nc.tensor.matmul(psA, eT[:, mc, qc * P:(qc + 1) * P],
                                     v_sb[:, mc, base:base + 257],
                                     start=(mc == 0), stop=(mc == NRT - 1))
                for mc in range(NRT):
                    nc.tensor.matmul(psB, eT[:, mc, qc * P:(qc + 1) * P],
                                     v_sb[:, mc, base + 257:base + VW],
                                     start=(mc == 0), stop=(mc == NRT - 1))
                rdq = stream.tile([P, 1], f32, tag="rd", bufs=2)
                nc.vector.reciprocal(out=rdq, in_=psA[:, 0:1])
                if hh == 0:
                    nc.scalar.activation(
                        out=oacc_qb[:, qc, 0:256], in_=psA[:, 1:257],
                        func=AF.Identity, scale=rdq)
                    nc.scalar.activation(
                        out=oacc_qb[:, qc, 256:512], in_=psB,
                        func=AF.Identity, scale=rdq)
                else:
                    rs_sb = stream.tile([P, D], bf16, tag="rs_sb", bufs=2)
                    nc.vector.scalar_tensor_tensor(
                        out=rs_sb[:, 0:256], in0=psA[:, 1:257], scalar=rdq,
                        in1=oacc_qb[:, qc, 0:256], op0=ALU.mult, op1=ALU.add)
                    nc.vector.scalar_tensor_tensor(
                        out=rs_sb[:, 256:512], in0=psB, scalar=rdq,
                        in1=oacc_qb[:, qc, 256:512], op0=ALU.mult, op1=ALU.add)
                    nc.sync.dma_start(out=rs_in[qb][qc * P:(qc + 1) * P, :],
                                      in_=rs_sb)

        def issue_rs(qb):
            if fake_rs:
                nc.sync.dma_start(out=rs_out[qb][:, :], in_=rs_in[qb][0:P, :])
            else:
                nc.gpsimd.collective_compute(
                    "ReduceScatter", ALU.add,
                    replica_groups=[[0, 1, 2, 3], [4, 5, 6, 7]],
                    ins=[rs_in[qb].opt()], outs=[rs_out[qb].opt()])

        def newton_rstd(mv, tag):
            """rstd = 1/sqrt(var+eps) on DVE; var+eps in [0.8, 1.21]."""
            v_t = stream.tile([P, 1], f32, tag=tag + "v")
            nc.vector.tensor_scalar(out=v_t, in0=mv[:, 1:2], scalar1=EPS,
                                    scalar2=None, op0=ALU.add)
            y = stream.tile([P, 1], f32, tag=tag + "y")
            nc.vector.tensor_scalar(out=y, in0=v_t, scalar1=-0.5, scalar2=1.5,
                                    op0=ALU.mult, op1=ALU.add)
            y2 = stream.tile([P, 1], f32, tag=tag + "y2")
            w = stream.tile([P, 1], f32, tag=tag + "w")
            for _ in range(2):
                nc.vector.tensor_tensor(out=y2, in0=y, in1=y, op=ALU.mult)
                nc.vector.tensor_tensor(out=y2, in0=v_t, in1=y2, op=ALU.mult)
                nc.vector.tensor_scalar(out=w, in0=y2, scalar1=-0.5, scalar2=1.5,
                                        op0=ALU.mult, op1=ALU.add)
                nc.vector.tensor_tensor(out=y, in0=y, in1=w, op=ALU.mult)
            return y

        def ffn(qbs):
            nb = len(qbs)
            xn2T = stream.tile([P, KC, nb * P], bf16, tag="xn2T", bufs=1)
            for j, qb in enumerate(qbs):
                rs_t = stream.tile([P, D], bf16, tag="rs_t", bufs=2)
                nc.sync.dma_start(out=rs_t, in_=rs_out[qb][:, :])
                xr_t = stream.tile([P, D], f32, tag="xr_t", bufs=2)
                nc.sync.dma_start(out=xr_t, in_=xrb[qb, :, :])
                nc.vector.tensor_tensor(out=x2_sb[:, qb, :], in0=rs_t,
                                        in1=xr_t, op=ALU.add)
                st6 = stream.tile([P, 6], f32, tag="st6c")
                nc.vector.bn_stats(out=st6, in_=x2_sb[:, qb, :])
                mv = stream.tile([P, 2], f32, tag="mvc")
                nc.vector.bn_aggr(out=mv, in_=st6)
                rstd = newton_rstd(mv, "nw")
                xn2_t = stream.tile([P, D], bf16, tag="xn2_t", bufs=2)
                nc.vector.tensor_scalar(out=xn2_t, in0=x2_sb[:, qb, :],
                                        scalar1=mv[:, 0:1], scalar2=rstd,
                                        op0=ALU.subtract, op1=ALU.mult)
                for kc in range(KC):
                    psT = psum.tile([P, P], bf16, tag="t", bufs=1)
                    nc.tensor.transpose(psT, xn2_t[:, kc * P:(kc + 1) * P], ident)
                    dst = xn2T[:, kc, j * P:(j + 1) * P]
                    if kc % 2 == 0:
                        nc.vector.tensor_copy(out=dst, in_=psT)
                    else:
                        nc.scalar.activation(out=dst, in_=psT, func=AF.Identity,
                                             scale=1.0)

            hT = stream.tile([P, EC, nb * P], bf16, tag="hT", bufs=1)
            for et in range(EC):
                ps_h = psum.tile([P, nb * P], f32, tag="avB", bufs=2)
                for kc in range(KC):
                    nc.tensor.matmul(ps_h, w1_sb[:, kc, et * P:(et + 1) * P],
                                     xn2T[:, kc, :],
                                     start=(kc == 0), stop=(kc == KC - 1))
                nc.scalar.activation(out=hT[:, et, :], in_=ps_h, func=AF.Silu,
                                     bias=b1_sb[:, et:et + 1], scale=1.0)

            for j, qb in enumerate(qbs):
                ps_o = psum.tile([P, D], f32, tag="avA", bufs=2)
                for ec in range(EC):
                    nc.tensor.matmul(ps_o, hT[:, ec, j * P:(j + 1) * P],
                                     w2_sb[:, ec, :],
                                     start=(ec == 0), stop=(ec == EC - 1))
                o_t = stream.tile([P, D], f32, tag="o_t", bufs=2)
                nc.vector.scalar_tensor_tensor(
                    out=o_t, in0=ps_o, scalar=1.0, in1=x2_sb[:, qb, :],
                    op0=ALU.mult, op1=ALU.add)
                nc.vector.tensor_tensor(out=o_t, in0=o_t, in1=b2_b, op=ALU.add)
                nc.sync.dma_start(out=out[qb * P:(qb + 1) * P, :], in_=o_t)

        if phases == "A":
            return
        if phases == "M":
            # PE microbench: av-like matmul chains at moving width W
            W = 512
            for f in flags:
                if f.startswith("w"):
                    W = int(f[1:])
            reuse_stat = "reuse" in flags
            dummy = main.tile([P, 512], bf16, name="dummy_m")
            for rep in range(8):
                for qc in range(4):
                    for half in range(512 // W):
                        ps = psum.tile([P, W], f32, tag="avA", bufs=2)
                        for mc in range(NRT):
                            st_mc = 0 if reuse_stat else mc
                            nc.tensor.matmul(
                                ps, v_sb[:, st_mc, qc * P:qc * P + P],
                                qkT[:, 0, half * W:(half + 1) * W],
                                start=(mc == 0), stop=(mc == NRT - 1))
                        nc.scalar.activation(out=dummy[:, :W], in_=ps,
                                             func=AF.Identity, scale=1.0)
            return
        units = [(qb, hh) for qb in range(NQB) for hh in range(H2)]
        if phases == "AB":
            ffn_sched = {}
        elif ffn_late:
            ffn_sched = {7: (0, 1, 2, 3)}
        else:
            ffn_sched = {5: (0, 1), 7: (2, 3)}
        oaccs = {}
        scores_exp(*units[0])
        for i, (qb, hh) in enumerate(units):
            if hh == 0:
                oaccs[qb] = stream.tile([P, 4, DV], bf16, tag="oacc", bufs=2,
                                        name=f"oacc{qb}")
            if i + 1 < len(units):
                scores_exp(*units[i + 1])
            attn_tail(qb, hh, oaccs[qb])
            if hh == 1:
                oaccs.pop(qb)
                issue_rs(qb)
            if i in ffn_sched:
                ffn(ffn_sched[i])


def build_nc(repeat=None, fake_rs=False, phases="ABC", flags=()):
    nc = bacc.Bacc("TRN2", target_bir_lowering=False, debug=False, num_devices=8)
    x = nc.dram_tensor("x", [N, D], bf16, kind="ExternalInput")
    xrb = nc.dram_tensor("xrb", [NQB, P, D], f32, kind="ExternalInput")
    wqkv = nc.dram_tensor("wqkv", [D, 2 * H2 * DH + H2 * DV], bf16,
                          kind="ExternalInput")
    bq_pt = nc.dram_tensor("bq_pt", [P, 1], f32, kind="ExternalInput")
    w1 = nc.dram_tensor("w1", [D, E], bf16, kind="ExternalInput")
    b1_pt = nc.dram_tensor("b1_pt", [P, EC], f32, kind="ExternalInput")
    w2 = nc.dram_tensor("w2", [E, D], bf16, kind="ExternalInput")
    b2 = nc.dram_tensor("b2", [1, D], f32, kind="ExternalInput")

    outs = {"out": nc.dram_tensor("out", [NQB * P, D], f32,
                                  kind="ExternalOutput").ap()}
    ins = (x.ap(), xrb.ap(), wqkv.ap(), bq_pt.ap(),
           w1.ap(), b1_pt.ap(), w2.ap(), b2.ap())
    with tile.TileContext(nc) as tc:
        if repeat:
            with tc.For_i(0, repeat, 1):
                build_body(tc, ins, outs, fake_rs=fake_rs, phases=phases,
                           flags=flags)
        else:
            build_body(tc, ins, outs, fake_rs=fake_rs, phases=phases,
                       flags=flags)
    nc.compile()
    return nc


def make_in_maps(inputs):
    """Host-side prep: fold LN gains into weights, drop the k-bias (cancels
    in softmax), fold Wv_h @ Wm_h per head, fold bv@Wm + bm into the
    residual rows, fold the softmax scale into the q weights."""
    from ml_dtypes import bfloat16 as np_bf16

    x = np.asarray(inputs["x"], np.float32)
    ln1_g = np.asarray(inputs["ln1_g"], np.float32)
    ln1_b = np.asarray(inputs["ln1_b"], np.float32)
    Wqkv = np.asarray(inputs["Wqkv"], np.float32)
    bqkv = np.asarray(inputs["bqkv"], np.float32)
    Wm = np.asarray(inputs["Wm"], np.float32)
    bm = np.asarray(inputs["bm"], np.float32)
    ln2_g = np.asarray(inputs["ln2_g"], np.float32)
    ln2_b = np.asarray(inputs["ln2_b"], np.float32)
    W1 = np.asarray(inputs["W1"], np.float32)
    b1 = np.asarray(inputs["b1"], np.float32)
    W2 = np.asarray(inputs["W2"], np.float32)
    b2 = np.asarray(inputs["b2"], np.float32)

    Wqkv_eff = ln1_g[:, None] * Wqkv
    bqkv_eff = ln1_b @ Wqkv + bqkv
    W1_eff = ln2_g[:, None] * W1
    b1_eff = ln2_b @ W1 + b1

    DQ = 512
    bv_full = bqkv_eff[2 * DQ:]
    bvwm = bv_full @ Wm
    Wv_eff = Wqkv_eff[:, 2 * DQ:].astype(np.float64)
    Wm64 = Wm.astype(np.float64)
    # per-head fold: v'_h = xn @ (Wv_h @ Wm_h)
    wfold = [np.asarray(Wv_eff[:, 512 * h:512 * (h + 1)]
                        @ Wm64[512 * h:512 * (h + 1), :], np.float32)
             for h in range(8)]

    in_maps = []
    for c in range(8):
        b = c // 4
        g = c % 4
        qcols = slice(DH * 2 * g, DH * 2 * g + 2 * DH)
        kcols = slice(DQ + DH * 2 * g, DQ + DH * 2 * g + 2 * DH)
        wqkv_c = np.concatenate(
            [Wqkv_eff[:, qcols] * SCALE, Wqkv_eff[:, kcols],
             wfold[2 * g], wfold[2 * g + 1]], axis=1)
        bq = bqkv_eff[qcols] * SCALE
        xrb = np.stack([x[b, QB * j + P * g:QB * j + P * (g + 1), :]
                        for j in range(NQB)]) \
            + bm[None, None, :] + bvwm[None, None, :]
        in_maps.append({
            "x": np.ascontiguousarray(x[b].astype(np_bf16)),
            "xrb": np.ascontiguousarray(xrb.astype(np.float32)),
            "wqkv": np.ascontiguousarray(wqkv_c.astype(np_bf16)),
            "bq_pt": np.ascontiguousarray(bq[:, None].astype(np.float32)),
            "w1": np.ascontiguousarray(W1_eff.astype(np_bf16)),
            "b1_pt": np.ascontiguousarray(
                b1_eff.reshape(EC, P).T.astype(np.float32)),
            "w2": np.ascontiguousarray(W2.astype(np_bf16)),
            "b2": np.ascontiguousarray(b2[None, :].astype(np.float32)),
        })
    return in_maps


def assemble_output(results):
    full = np.empty((2, N, D), np.float32)
    for c in range(8):
        b, rank = c // 4, c % 4
        o = results[c]["out"]
        for j in range(NQB):
            full[b, QB * j + P * rank:QB * j + P * (rank + 1), :] = \
                o[P * j:P * (j + 1), :]
    return full

_NC_CACHE = {}


def kernel(**inputs) -> np.ndarray:
    key = "nc8"
    if key not in _NC_CACHE:
        _NC_CACHE[key] = build_nc()
    nc = _NC_CACHE[key]
    in_maps = make_in_maps(inputs)
    res = bass_utils.run_bass_kernel_spmd(nc, in_maps, core_ids=list(range(8)))
    return assemble_output(res.results)


# revision 5
# speedup vs baseline: 1.3267x; 1.0825x over previous
"""AttentionBlock Trainium2 kernel — 8-core SPMD, v7.

vs v6 (402us -> ~305us fake-RS amplified estimate):
- Wv@Wm folded on host per head: v' = xn @ (Wv_h Wm_h) so the merge
  matmul disappears; head outputs are summed after per-head softmax
  normalization (rows of A sum to 1, so bv@Wm still folds into the
  residual rows).
- attn@v uses eT chunks as the stationary operand, producing [q, dv]
  directly (the layout the ReduceScatter wants) — no output transpose,
  and the normalization becomes a per-partition tensor_scalar.
- The softmax denominator rides along as a leading ones-column in the
  v' moving operand (PSUM col 0 accumulates sum_k exp), killing the
  separate ones-matmul denominator chain and the bc broadcast.
- Score kt-pairs share one 2-bank [P,1024] PSUM tile so each exp op
  drains 1024 cols; per-op fixed cost (~200ns) measured equal on
  Act and DVE, so exp alternates pairs between them.
- LN2 rstd via 2-round Newton on DVE (var+eps in [0.8, 1.21]) so the
  Act table never leaves exp<->silu; fewer 1.28us table reloads.
- Phase A LN chain software-pipelined (stats of rt+1 emitted before
  the finish half of rt) so the DVE never head-of-line blocks on the
  Act sqrt round-trip.
- FFN split into front (x2+LN2+FF1+silu) and back (FF2+out) halves,
  interleaved at separate attention-unit boundaries so each block
  injects less Act/DVE work in one place.
- GPSIMD (Pool) is compute-useless here (~10x slower than spec for
  elementwise, cannot touch PSUM) — used only for weight-DMA queueing.
"""

import numpy as np
import concourse.bass as bass
import concourse.bacc as bacc
import concourse.mybir as mybir
import concourse.tile as tile
from concourse import bass_utils
from concourse.masks import make_identity

P = 128
N = 2048
D = 512
H2 = 2
DH = 64
DV = 512           # folded per-head value dim == D
VW = DV + 1        # [ones | v'] per head
E = 2048
QB = 512
NQB = N // QB      # 4
NRT = N // P       # 16
KC = D // P        # 4
EC = E // P        # 16
EPS = 1e-5
SCALE = DH ** -0.5

f32 = mybir.dt.float32
bf16 = mybir.dt.bfloat16
i16 = mybir.dt.int16
ET_BUFS = 2
A16 = 184.6650085170266   # 2^7/ln2
B16 = 16249.0             # 127*2^7 + schraudolph offset (tuned)

AF = mybir.ActivationFunctionType
ALU = mybir.AluOpType


def build_body(tc, ins, outs, fake_rs=False, phases="ABC", flags=()):
    nc = tc.nc
    x, xrb, wqkv, bq_pt, w1, b1_pt, w2, b2_d = ins
    out = outs["out"]
    exp2 = "exp2" in flags            # v6-style 2-way exp split
    ffn_late = "ffnlate" in flags     # all FFN at the end
    norm_dve = "normact" not in flags  # h0 softmax normalize on DVE
    depth1 = "depth1" in flags        # 1-unit score lookahead (not 2)

    import contextlib
    est = contextlib.ExitStack()
    with est:
        const = est.enter_context(tc.tile_pool(name="const", bufs=1))
        dram = est.enter_context(tc.tile_pool(name="dram", bufs=1, space="DRAM"))
        main = est.enter_context(tc.tile_pool(name="main", bufs=1))
        stream = est.enter_context(tc.tile_pool(name="stream", bufs=3))
        psum = est.enter_context(tc.tile_pool(name="psum", bufs=1, space="PSUM"))

        # ---- constants ----
        ident_f = const.tile([P, P], f32)
        make_identity(nc, ident_f)
        ident = const.tile([P, P], bf16)
        nc.vector.tensor_copy(ident, ident_f)
        eps_t = const.tile([P, 1], f32)
        nc.vector.memset(eps_t, EPS)

        bq_sb = const.tile([P, 1], f32)
        nc.sync.dma_start(out=bq_sb, in_=bq_pt[:, :])
        b1_sb = const.tile([P, EC], f32)
        nc.sync.dma_start(out=b1_sb, in_=b1_pt[:, :])
        b2_b = const.tile([P, D], f32)
        nc.gpsimd.dma_start(
            out=b2_b,
            in_=bass.AP(tensor=b2_d.tensor, offset=b2_d.offset, ap=[[0, P], [1, D]]))

        rs_in = [dram.tile([QB, D], bf16, name=f"rs_in{j}", tag=f"rs_in{j}")
                 for j in range(NQB)]
        rs_out = [dram.tile([P, D], bf16, name=f"rs_out{j}", tag=f"rs_out{j}")
                  for j in range(NQB)]

        # ---- persistent SBUF tiles (weights on the gpsimd SW queue) ----
        wqkv_sb = main.tile([P, KC, 2 * H2 * DH + H2 * DV], bf16)
        nc.gpsimd.dma_start(out=wqkv_sb, in_=wqkv.rearrange("(c p) n -> p c n", p=P))
        w1_sb = main.tile([P, KC, E], bf16)
        w1r = w1.rearrange("(c p) n -> p c n", p=P)
        for kc in range(KC):
            nc.gpsimd.dma_start(out=w1_sb[:, kc, :], in_=w1r[:, kc, :])
        w2_sb = main.tile([P, EC, D], bf16)
        w2r = w2.rearrange("(c p) n -> p c n", p=P)
        for j in range(4):
            nc.gpsimd.dma_start(out=w2_sb[:, 4 * j:4 * (j + 1), :],
                                in_=w2r[:, 4 * j:4 * (j + 1), :])

        qkT = main.tile([P, 2, N], bf16)
        v_sb = main.tile([P, NRT, H2 * VW], bf16)
        x2_sb = main.tile([P, NQB, D], f32)

        # ones columns of v' (denominator rider)
        for hh in range(H2):
            nc.vector.memset(v_sb[:, :, hh * VW:hh * VW + 1], 1.0)

        # ---------------- Phase A: LN1 + xbar transposes + qkv ----------------
        # Software-pipelined: the stats half of LN for rt+1 is emitted
        # before the finish half (recip/xn/transposes/v') of rt, so the
        # DVE never head-of-line blocks on the Act sqrt round-trip.
        a_v6 = "a_v6" in flags            # v6 ordering (LN loop, qk, v loop)

        def ln_start(rt):
            x_t = stream.tile([P, D], bf16, tag="x_t", bufs=3)
            nc.sync.dma_start(out=x_t, in_=x[rt * P:(rt + 1) * P, :])
            st6 = stream.tile([P, 6], f32, tag="st6")
            nc.vector.bn_stats(out=st6, in_=x_t)
            mv = stream.tile([P, 2], f32, tag="mv", bufs=2)
            nc.vector.bn_aggr(out=mv, in_=st6)
            sd = stream.tile([P, 1], f32, tag="sd", bufs=2)
            nc.scalar.activation(out=sd, in_=mv[:, 1:2], func=AF.Sqrt,
                                 bias=eps_t, scale=1.0)
            return x_t, mv, sd

        def ln_finish(rt, st, xnT, defer_copies=False):
            x_t, mv, sd = st
            rstd = stream.tile([P, 1], f32, tag="rstd")
            nc.vector.reciprocal(out=rstd, in_=sd)
            xn_t = stream.tile([P, D], bf16, tag="xn_t", bufs=3)
            nc.vector.tensor_scalar(out=xn_t, in0=x_t,
                                    scalar1=mv[:, 0:1], scalar2=rstd,
                                    op0=ALU.subtract, op1=ALU.mult)
            for kc in range(KC):
                psT = psum.tile([P, P], bf16, tag="avB", bufs=2,
                                name=f"psT_{rt}_{kc}")
                nc.tensor.transpose(psT, xn_t[:, kc * P:(kc + 1) * P], ident)
                nc.vector.tensor_copy(out=xnT[:, kc, rt * P:(rt + 1) * P],
                                      in_=psT)

        def vproj_rt(rt, xnT):
            for cr in range(H2):
                ps_v = psum.tile([P, DV], f32, tag="avA", bufs=2)
                for kc in range(KC):
                    nc.tensor.matmul(
                        ps_v, xnT[:, kc, rt * P:(rt + 1) * P],
                        wqkv_sb[:, kc, 2 * H2 * DH + cr * DV:
                                2 * H2 * DH + (cr + 1) * DV],
                        start=(kc == 0), stop=(kc == KC - 1))
                nc.scalar.activation(out=v_sb[:, rt, cr * VW + 1:(cr + 1) * VW],
                                     in_=ps_v, func=AF.Identity, scale=1.0)

        def qk_rr(rr, xnT):
            for ct in range(2):
                ps_qk = psum.tile([P, QB], f32, tag="s", bufs=2)
                for kc in range(KC):
                    nc.tensor.matmul(
                        ps_qk, wqkv_sb[:, kc, ct * P:(ct + 1) * P],
                        xnT[:, kc, rr * QB:(rr + 1) * QB],
                        start=(kc == 0), stop=(kc == KC - 1))
                nc.scalar.activation(
                    out=qkT[:, ct, rr * QB:(rr + 1) * QB], in_=ps_qk,
                    func=AF.Identity,
                    bias=(bq_sb if ct == 0 else 0.0), scale=1.0)

        def phase_a(pre_scores=None):
            with tc.tile_pool(name="poolA", bufs=1) as poolA:
                xnT = poolA.tile([P, KC, N], bf16)
                st = ln_start(0)
                for rt in range(NRT):
                    nxt = ln_start(rt + 1) if rt + 1 < NRT else None
                    ln_finish(rt, st, xnT)
                    st = nxt
                for rr in range(NQB):
                    qk_rr(rr, xnT)
                # first two score blocks go here: they only need qkT, and
                # their exp drains overlap the v' projections below
                if pre_scores:
                    pre_scores()
                for rt in range(NRT):
                    vproj_rt(rt, xnT)

        # ---------------- Phase B/C interleaved ----------------
        unit_state = {}

        def scores_exp(qb, hh):
            """Scores kt-pairs share one 2-bank PSUM tile so each exp op
            drains 1024 cols — halves the per-op fixed overhead."""
            hp = slice(DH * hh, DH * (hh + 1))
            eT = stream.tile([P, NRT, QB], bf16, tag="eT", bufs=ET_BUFS,
                             name=f"eT_{qb}_{hh}")
            for k2 in range(NRT // 2):
                ps2 = psum.tile([P, 2 * QB], f32, tag="s", bufs=2)
                for j in range(2):
                    kt = 2 * k2 + j
                    nc.tensor.matmul(
                        ps2[:, j * QB:(j + 1) * QB],
                        qkT[hp, 1, kt * P:(kt + 1) * P],
                        qkT[hp, 0, qb * QB:(qb + 1) * QB],
                        start=True, stop=True)
                dst = eT[:, 2 * k2:2 * k2 + 2, :]
                if k2 % 2 == 0:
                    # scale pre-folded into q on the host
                    nc.scalar.activation(out=dst, in_=ps2, func=AF.Exp,
                                         scale=1.0)
                else:
                    # Schraudolph exp: bf16 bits = s*2^7/ln2 + B16
                    with nc.allow_low_precision(reason="schraudolph exp"):
                        nc.vector.tensor_scalar(
                            out=dst.bitcast(i16), in0=ps2,
                            scalar1=A16, scalar2=B16,
                            op0=ALU.mult, op1=ALU.add)
            unit_state[(qb, hh)] = eT

        def attn_tail(qb, hh, oacc_qb):
            """A@v' in [q, dv] layout with denominator in PSUM col 0;
            per-head normalize via per-partition scalar; heads summed."""
            eT = unit_state.pop((qb, hh))
            base = hh * VW
            for qc in range(4):
                psA = psum.tile([P, 257], f32, tag="avA", bufs=2)
                psB = psum.tile([P, 256], f32, tag="avB", bufs=2)
                for mc in range(NRT):
                    nc.tensor.matmul(psA, eT[:, mc, qc * P:(qc + 1) * P],
                                     v_sb[:, mc, base:base + 257],
                                     start=(mc == 0), stop=(mc == NRT - 1))
                for mc in range(NRT):
                    nc.tensor.matmul(psB, eT[:, mc, qc * P:(qc + 1) * P],
                                     v_sb[:, mc, base + 257:base + VW],
                                     start=(mc == 0), stop=(mc == NRT - 1))
                rdq = stream.tile([P, 1], f32, tag="rd", bufs=2)
                nc.vector.reciprocal(out=rdq, in_=psA[:, 0:1])
                if hh == 0:
                    if norm_dve:
                        nc.vector.tensor_scalar(
                            out=oacc_qb[:, qc, 0:256], in0=psA[:, 1:257],
                            scalar1=rdq, scalar2=None, op0=ALU.mult)
                        nc.vector.tensor_scalar(
                            out=oacc_qb[:, qc, 256:512], in0=psB,
                            scalar1=rdq, scalar2=None, op0=ALU.mult)
                    else:
                        nc.scalar.activation(
                            out=oacc_qb[:, qc, 0:256], in_=psA[:, 1:257],
                            func=AF.Identity, scale=rdq)
                        nc.scalar.activation(
                            out=oacc_qb[:, qc, 256:512], in_=psB,
                            func=AF.Identity, scale=rdq)
                else:
                    rs_sb = stream.tile([P, D], bf16, tag="rs_sb", bufs=2)
                    nc.vector.scalar_tensor_tensor(
                        out=rs_sb[:, 0:256], in0=psA[:, 1:257], scalar=rdq,
                        in1=oacc_qb[:, qc, 0:256], op0=ALU.mult, op1=ALU.add)
                    nc.vector.scalar_tensor_tensor(
                        out=rs_sb[:, 256:512], in0=psB, scalar=rdq,
                        in1=oacc_qb[:, qc, 256:512], op0=ALU.mult, op1=ALU.add)
                    nc.sync.dma_start(out=rs_in[qb][qc * P:(qc + 1) * P, :],
                                      in_=rs_sb)

        def issue_rs(qb):
            if fake_rs:
                nc.sync.dma_start(out=rs_out[qb][:, :], in_=rs_in[qb][0:P, :])
            else:
                nc.gpsimd.collective_compute(
                    "ReduceScatter", ALU.add,
                    replica_groups=[[0, 1, 2, 3], [4, 5, 6, 7]],
                    ins=[rs_in[qb].opt()], outs=[rs_out[qb].opt()])

        def newton_rstd(mv, tag):
            """rstd = 1/sqrt(var+eps) on DVE; var+eps in [0.8, 1.21]."""
            v_t = stream.tile([P, 1], f32, tag=tag + "v")
            nc.vector.tensor_scalar(out=v_t, in0=mv[:, 1:2], scalar1=EPS,
                                    scalar2=None, op0=ALU.add)
            y = stream.tile([P, 1], f32, tag=tag + "y")
            nc.vector.tensor_scalar(out=y, in0=v_t, scalar1=-0.5, scalar2=1.5,
                                    op0=ALU.mult, op1=ALU.add)
            y2 = stream.tile([P, 1], f32, tag=tag + "y2")
            w = stream.tile([P, 1], f32, tag=tag + "w")
            for _ in range(2):
                nc.vector.tensor_tensor(out=y2, in0=y, in1=y, op=ALU.mult)
                nc.vector.tensor_tensor(out=y2, in0=v_t, in1=y2, op=ALU.mult)
                nc.vector.tensor_scalar(out=w, in0=y2, scalar1=-0.5, scalar2=1.5,
                                        op0=ALU.mult, op1=ALU.add)
                nc.vector.tensor_tensor(out=y, in0=y, in1=w, op=ALU.mult)
            return y

        ffn_state = {}

        def ffn_front(qb):
            """x2, LN2, transposes, FF1+silu for one q-block."""
            xn2T = stream.tile([P, KC, P], bf16, tag="xn2T", bufs=1,
                               name=f"xn2T_{qb}")
            rs_t = stream.tile([P, D], bf16, tag="rs_t", bufs=2)
            nc.sync.dma_start(out=rs_t, in_=rs_out[qb][:, :])
            xr_t = stream.tile([P, D], f32, tag="xr_t", bufs=2)
            nc.sync.dma_start(out=xr_t, in_=xrb[qb, :, :])
            nc.vector.tensor_tensor(out=x2_sb[:, qb, :], in0=rs_t,
                                    in1=xr_t, op=ALU.add)
            st6 = stream.tile([P, 6], f32, tag="st6c")
            nc.vector.bn_stats(out=st6, in_=x2_sb[:, qb, :])
            mv = stream.tile([P, 2], f32, tag="mvc")
            nc.vector.bn_aggr(out=mv, in_=st6)
            rstd = newton_rstd(mv, "nw")
            xn2_t = stream.tile([P, D], bf16, tag="xn2_t", bufs=2)
            nc.vector.tensor_scalar(out=xn2_t, in0=x2_sb[:, qb, :],
                                    scalar1=mv[:, 0:1], scalar2=rstd,
                                    op0=ALU.subtract, op1=ALU.mult)
            for kc in range(KC):
                psT = psum.tile([P, P], bf16, tag="avB", bufs=2)
                nc.tensor.transpose(psT, xn2_t[:, kc * P:(kc + 1) * P], ident)
                dst = xn2T[:, kc, :]
                if kc % 2 == 0:
                    nc.vector.tensor_copy(out=dst, in_=psT)
                else:
                    nc.scalar.activation(out=dst, in_=psT, func=AF.Identity,
                                         scale=1.0)
            hT = stream.tile([P, EC, P], bf16, tag="hT", bufs=1,
                             name=f"hT_{qb}")
            for et in range(EC):
                ps_h = psum.tile([P, P], f32, tag="avB", bufs=2)
                for kc in range(KC):
                    nc.tensor.matmul(ps_h, w1_sb[:, kc, et * P:(et + 1) * P],
                                     xn2T[:, kc, :],
                                     start=(kc == 0), stop=(kc == KC - 1))
                nc.scalar.activation(out=hT[:, et, :], in_=ps_h, func=AF.Silu,
                                     bias=b1_sb[:, et:et + 1], scale=1.0)
            ffn_state[qb] = hT

        def ffn_back(qb):
            hT = ffn_state.pop(qb)
            ps_o = psum.tile([P, D], f32, tag="avA", bufs=2)
            for ec in range(EC):
                nc.tensor.matmul(ps_o, hT[:, ec, :], w2_sb[:, ec, :],
                                 start=(ec == 0), stop=(ec == EC - 1))
            o_t = stream.tile([P, D], f32, tag="o_t", bufs=2)
            nc.vector.scalar_tensor_tensor(
                out=o_t, in0=ps_o, scalar=1.0, in1=x2_sb[:, qb, :],
                op0=ALU.mult, op1=ALU.add)
            nc.vector.tensor_tensor(out=o_t, in0=o_t, in1=b2_b, op=ALU.add)
            nc.sync.dma_start(out=out[qb * P:(qb + 1) * P, :], in_=o_t)

        def ffn(qbs):
            for qb in qbs:
                ffn_front(qb)
                ffn_back(qb)

        if phases == "A":
            phase_a()
            return
        if phases == "E":
            # engine op-cost microbench: 256 reps of one op kind
            phase_a()
            kind = [f for f in flags if f.startswith("e_")][0]
            src_ps = psum.tile([P, 512], f32, tag="s", bufs=2)
            nc.tensor.matmul(src_ps, qkT[0:64, 1, 0:P], qkT[0:64, 0, 0:512],
                             start=True, stop=True)
            for i in range(256):
                dst = stream.tile([P, 512], bf16, tag="edst", bufs=4)
                if kind == "e_act_id":
                    nc.scalar.activation(out=dst, in_=src_ps, func=AF.Identity,
                                         scale=1.0)
                elif kind == "e_act_exp":
                    nc.scalar.activation(out=dst, in_=src_ps, func=AF.Exp,
                                         scale=1.0)
                elif kind == "e_act_bias":
                    nc.scalar.activation(out=dst, in_=src_ps, func=AF.Identity,
                                         bias=eps_t, scale=eps_t)
                elif kind == "e_dve_copy":
                    nc.vector.tensor_copy(out=dst, in_=src_ps)
                elif kind == "e_dve_ts":
                    with nc.allow_low_precision(reason="bench"):
                        nc.vector.tensor_scalar(
                            out=dst.bitcast(i16), in0=src_ps,
                            scalar1=A16, scalar2=B16,
                            op0=ALU.mult, op1=ALU.add)
                elif kind == "e_dve_stt":
                    nc.vector.scalar_tensor_tensor(
                        out=dst, in0=src_ps, scalar=1.0, in1=b2_b,
                        op0=ALU.mult, op1=ALU.add)
                elif kind == "e_act_sb":
                    nc.scalar.activation(out=dst, in_=b2_b, func=AF.Identity,
                                         scale=1.0)
            return
        if phases == "M":
            phase_a()
            # PE microbench: av-like matmul chains at moving width W
            W = 512
            for f in flags:
                if f.startswith("w"):
                    W = int(f[1:])
            reuse_stat = "reuse" in flags
            dummy = main.tile([P, 512], bf16, name="dummy_m")
            for rep in range(8):
                for qc in range(4):
                    for half in range(512 // W):
                        ps = psum.tile([P, W], f32, tag="avA", bufs=2)
                        for mc in range(NRT):
                            st_mc = 0 if reuse_stat else mc
                            nc.tensor.matmul(
                                ps, v_sb[:, st_mc, qc * P:qc * P + P],
                                qkT[:, 0, half * W:(half + 1) * W],
                                start=(mc == 0), stop=(mc == NRT - 1))
                        nc.scalar.activation(out=dummy[:, :W], in_=ps,
                                             func=AF.Identity, scale=1.0)
            return
        units = [(qb, hh) for qb in range(NQB) for hh in range(H2)]
        if phases == "AB":
            ffn_sched = {}
        elif "ffn22" in flags:
            ffn_sched = {5: (0, 1), 7: (2, 3)}
        elif "ffn4" in flags:
            ffn_sched = {2: (0,), 4: (1,), 6: (2,), 7: (3,)}
        elif ffn_late:
            ffn_sched = {7: (0, 1, 2, 3)}
        else:
            # split fronts (LN2+FF1+silu) and backs (FF2+out) across unit
            # boundaries so each block injects less Act/DVE work at once
            ffn_sched = {2: (("F", 0),), 3: (("B", 0),), 4: (("F", 1),),
                         5: (("B", 1),), 6: (("F", 2),),
                         7: (("B", 2), ("F", 3), ("B", 3))}
        oaccs = {}

        def sched_scores(i):
            qb, hh = units[i]
            if hh == 0:
                oaccs[qb] = stream.tile([P, 4, DV], bf16, tag="oacc", bufs=2,
                                        name=f"oacc{qb}")
            scores_exp(qb, hh)

        phase_a()
        sched_scores(0)
        for i, (qb, hh) in enumerate(units):
            if i + 1 < len(units):
                sched_scores(i + 1)
            attn_tail(qb, hh, oaccs[qb])
            if hh == 1:
                oaccs.pop(qb)
                issue_rs(qb)
            for item in ffn_sched.get(i, ()):
                if isinstance(item, tuple):
                    (ffn_front if item[0] == "F" else ffn_back)(item[1])
                else:
                    ffn((item,))


def build_nc(repeat=None, fake_rs=False, phases="ABC", flags=()):
    nc = bacc.Bacc("TRN2", target_bir_lowering=False, debug=False, num_devices=8)
    x = nc.dram_tensor("x", [N, D], bf16, kind="ExternalInput")
    xrb = nc.dram_tensor("xrb", [NQB, P, D], f32, kind="ExternalInput")
    wqkv = nc.dram_tensor("wqkv", [D, 2 * H2 * DH + H2 * DV], bf16,
                          kind="ExternalInput")
    bq_pt = nc.dram_tensor("bq_pt", [P, 1], f32, kind="ExternalInput")
    w1 = nc.dram_tensor("w1", [D, E], bf16, kind="ExternalInput")
    b1_pt = nc.dram_tensor("b1_pt", [P, EC], f32, kind="ExternalInput")
    w2 = nc.dram_tensor("w2", [E, D], bf16, kind="ExternalInput")
    b2 = nc.dram_tensor("b2", [1, D], f32, kind="ExternalInput")

    outs = {"out": nc.dram_tensor("out", [NQB * P, D], f32,
                                  kind="ExternalOutput").ap()}
    ins = (x.ap(), xrb.ap(), wqkv.ap(), bq_pt.ap(),
           w1.ap(), b1_pt.ap(), w2.ap(), b2.ap())
    with tile.TileContext(nc) as tc:
        if repeat:
            with tc.For_i(0, repeat, 1):
                build_body(tc, ins, outs, fake_rs=fake_rs, phases=phases,
                           flags=flags)
        else:
            build_body(tc, ins, outs, fake_rs=fake_rs, phases=phases,
                       flags=flags)
    nc.compile()
    return nc


def make_in_maps(inputs):
    """Host-side prep: fold LN gains into weights, drop the k-bias (cancels
    in softmax), fold Wv_h @ Wm_h per head, fold bv@Wm + bm into the
    residual rows, fold the softmax scale into the q weights."""
    from ml_dtypes import bfloat16 as np_bf16

    x = np.asarray(inputs["x"], np.float32)
    ln1_g = np.asarray(inputs["ln1_g"], np.float32)
    ln1_b = np.asarray(inputs["ln1_b"], np.float32)
    Wqkv = np.asarray(inputs["Wqkv"], np.float32)
    bqkv = np.asarray(inputs["bqkv"], np.float32)
    Wm = np.asarray(inputs["Wm"], np.float32)
    bm = np.asarray(inputs["bm"], np.float32)
    ln2_g = np.asarray(inputs["ln2_g"], np.float32)
    ln2_b = np.asarray(inputs["ln2_b"], np.float32)
    W1 = np.asarray(inputs["W1"], np.float32)
    b1 = np.asarray(inputs["b1"], np.float32)
    W2 = np.asarray(inputs["W2"], np.float32)
    b2 = np.asarray(inputs["b2"], np.float32)

    Wqkv_eff = ln1_g[:, None] * Wqkv
    bqkv_eff = ln1_b @ Wqkv + bqkv
    W1_eff = ln2_g[:, None] * W1
    b1_eff = ln2_b @ W1 + b1

    DQ = 512
    bv_full = bqkv_eff[2 * DQ:]
    bvwm = bv_full @ Wm
    Wv_eff = Wqkv_eff[:, 2 * DQ:].astype(np.float64)
    Wm64 = Wm.astype(np.float64)
    # per-head fold: v'_h = xn @ (Wv_h @ Wm_h)
    wfold = [np.asarray(Wv_eff[:, 512 * h:512 * (h + 1)]
                        @ Wm64[512 * h:512 * (h + 1), :], np.float32)
             for h in range(8)]

    in_maps = []
    for c in range(8):
        b = c // 4
        g = c % 4
        qcols = slice(DH * 2 * g, DH * 2 * g + 2 * DH)
        kcols = slice(DQ + DH * 2 * g, DQ + DH * 2 * g + 2 * DH)
        wqkv_c = np.concatenate(
            [Wqkv_eff[:, qcols] * SCALE, Wqkv_eff[:, kcols],
             wfold[2 * g], wfold[2 * g + 1]], axis=1)
        bq = bqkv_eff[qcols] * SCALE
        xrb = np.stack([x[b, QB * j + P * g:QB * j + P * (g + 1), :]
                        for j in range(NQB)]) \
            + bm[None, None, :] + bvwm[None, None, :]
        in_maps.append({
            "x": np.ascontiguousarray(x[b].astype(np_bf16)),
            "xrb": np.ascontiguousarray(xrb.astype(np.float32)),
            "wqkv": np.ascontiguousarray(wqkv_c.astype(np_bf16)),
            "bq_pt": np.ascontiguousarray(bq[:, None].astype(np.float32)),
            "w1": np.ascontiguousarray(W1_eff.astype(np_bf16)),
            "b1_pt": np.ascontiguousarray(
                b1_eff.reshape(EC, P).T.astype(np.float32)),
            "w2": np.ascontiguousarray(W2.astype(np_bf16)),
            "b2": np.ascontiguousarray(b2[None, :].astype(np.float32)),
        })
    return in_maps


def assemble_output(results):
    full = np.empty((2, N, D), np.float32)
    for c in range(8):
        b, rank = c // 4, c % 4
        o = results[c]["out"]
        for j in range(NQB):
            full[b, QB * j + P * rank:QB * j + P * (rank + 1), :] = \
                o[P * j:P * (j + 1), :]
    return full

_NC_CACHE = {}


def kernel(**inputs) -> np.ndarray:
    key = "nc8"
    if key not in _NC_CACHE:
        _NC_CACHE[key] = build_nc()
    nc = _NC_CACHE[key]
    in_maps = make_in_maps(inputs)
    res = bass_utils.run_bass_kernel_spmd(nc, in_maps, core_ids=list(range(8)))
    return assemble_output(res.results)
